# revision 11
# baseline (speedup 1.0000x reference)
# Trainium2 Bass kernel for nn_Attention_65609920413963 (sparse block-masked attention).
#
# Math structure exploited (verified against the reference numerics):
#   L_b = n1[b]*n2[b].  The reference writes NEG=-1e10 into masked logits and
#   then adds K (|K| < 1), which rounds to exactly -1e10 in fp32.  Hence:
#     * rows >= L_b: every logit is exactly -1e10 -> softmax is exactly uniform
#       -> out_row = mean(V) @ proj_w.T + proj_b  (identical for all such rows;
#       computed on host, it is O(N*C) work).
#     * rows < L_b: masked cols underflow to exp(.)=0 exactly -> softmax over
#       cols < L_b only, with additive bias K[b,row,col] on the active logits.
#   Device computes only the active [0:PAD) x [0:PAD) region (PAD >= max L,
#   multiple of 128).
#
# Sharding: 8 cores = (batch b in 0..3) x (head-half g in 0..1, 8 heads each).
# Per-core device pipeline (all matmuls fp32r):
#   QT/KT  [ch, rows]   = Wq/Wk.T @ x.T          (feature-major)
#   V      [keys, ch]   = x @ Wv                 (row-major)
#   ST_h   [keys, rows] = KB + K8_h @ Q_h.T      (KB = K^T with masked cols at
#                                                 -200, injected into PSUM by an
#                                                 identity matmul so the ST
#                                                 accumulation starts from the
#                                                 bias; masked cols underflow to
#                                                 exp 0 with no extra masking)
#   PT_h   = exp(ST_h)                           (ACT reads PSUM directly)
#   OT_h   [.., rows]   = [V_h | ones].T @ PT_h  (ones column -> partition 64/0
#                                                 carries the softmax denominators)
#   OTn_h  = OT_h * (1/denominator)              (1/den broadcast across the 64
#                                                 col partitions by a K=1 matmul
#                                                 against an all-ones lhsT)
#   Y      [rows, 1024] = OTn @ proj_w_g         (partial product; host adds the
#                                                 two head-halves + proj_b)
import numpy as np

B, N, C = 4, 1024, 1024
H, Dh = 16, 64
HG = H // 2          # heads per core
GC = HG * Dh         # channels per core (512)
NCC = C // 128       # 8 contraction chunks

_CACHE = {}


def _build_program(PAD, reps=1, n_j=4, do_y=True):
    import concourse.bacc as bacc
    import concourse.bass as bass
    import concourse.mybir as mybir
    import concourse.tile as tile

    NT = PAD // 128
    HCH = PAD // 2    # psum half-chunk of the row dimension (<=512, >=256)
    assert 256 <= HCH <= 512

    F32 = mybir.dt.float32
    F32R = mybir.dt.float32r
    F16 = mybir.dt.float16

    nc = bacc.Bacc("TRN2", target_bir_lowering=False, debug=False)

    xt_d = nc.dram_tensor("xt", [C, PAD], F16, kind="ExternalInput")
    wq_d = nc.dram_tensor("wq", [C, GC], F16, kind="ExternalInput")
    wk_d = nc.dram_tensor("wk", [C, GC], F16, kind="ExternalInput")
    wv_d = nc.dram_tensor("wv", [C, GC], F16, kind="ExternalInput")
    pw_d = nc.dram_tensor("pw", [GC, C], F16, kind="ExternalInput")
    kb_d = nc.dram_tensor("kb", [PAD, PAD], F16, kind="ExternalInput")
    id_d = nc.dram_tensor("ident", [128, 128], F16, kind="ExternalInput")
    on_d = nc.dram_tensor("onesb", [128, 128], F32R, kind="ExternalInput")
    y_d = nc.dram_tensor("y", [PAD, C], F32, kind="ExternalOutput")

    def r(ap):
        return ap

    import contextlib

    with tile.TileContext(nc) as tc:
        with (
            tc.For_i(0, reps, 1) if reps > 1 else contextlib.nullcontext(),
            tc.tile_pool(name="singles", bufs=1) as singles,
            tc.tile_pool(name="wpool", bufs=2) as wpool,
            tc.tile_pool(name="work", bufs=3) as work,
            tc.tile_pool(name="ptpool", bufs=3) as ptpool,
            tc.tile_pool(name="psA", bufs=2, space="PSUM") as psA,
            tc.tile_pool(name="psB", bufs=1, space="PSUM") as psB,
        ):
            # ---- resident SBUF tensors -------------------------------------
            xt_sb = singles.tile([128, NCC, PAD], F16, tag="xt")
            # wq/wk/wv share 2 slots: wv reuses wq's slot once QT is done
            wq_sb = wpool.tile([128, NCC, GC], F16, tag="w")
            wk_sb = wpool.tile([128, NCC, GC], F16, tag="w")
            wv_sb = wpool.tile([128, NCC, GC], F16, tag="w")
            pw_sb = singles.tile([128, 4, C], F16, tag="pw")
            kb_sb = singles.tile([128, NT, PAD], F16, tag="kb")
            id_sb = singles.tile([128, 128], F16, tag="id")
            on_sb = singles.tile([128, 128], F32R, tag="on")
            qt_sb = singles.tile([128, 4, PAD], F32R, tag="qt")
            kt_sb = singles.tile([128, 4, PAD], F32R, tag="kt")
            vp_sb = singles.tile([128, NT, HG, 128], F32R, tag="vp")
            otn_sb = singles.tile([128, 4, PAD], F16, tag="otn")

            # per-contraction-chunk DMAs, interleaved so the first QT matmuls
            # start after ~0.7MB instead of the full 5MB of xt+wq
            xt_r = xt_d.ap().rearrange("(a p) r -> p a r", p=128)
            wq_r = wq_d.ap().rearrange("(a p) m -> p a m", p=128)
            wk_r = wk_d.ap().rearrange("(a p) m -> p a m", p=128)
            for cc in range(NCC):
                nc.sync.dma_start(out=xt_sb[:, cc], in_=xt_r[:, cc])
                nc.sync.dma_start(out=wq_sb[:, cc], in_=wq_r[:, cc])
            for cc in range(NCC):
                nc.sync.dma_start(out=wk_sb[:, cc], in_=wk_r[:, cc])
            nc.sync.dma_start(out=wv_sb, in_=wv_d.ap().rearrange("(a p) m -> p a m", p=128))
            nc.sync.dma_start(out=kb_sb, in_=kb_d.ap().rearrange("(t p) r -> p t r", p=128))
            nc.sync.dma_start(out=id_sb, in_=id_d.ap())
            nc.sync.dma_start(out=on_sb, in_=on_d.ap())
            nc.sync.dma_start(out=pw_sb, in_=pw_d.ap().rearrange("(j p) o -> p j o", p=128))

            # vp: [keypart, keytile, head, Mcol].  even h: cols 0..63 = V_h, col 64 = 1
            #                                      odd  h: col 0 = 1, cols 64..127 = V_h
            zb_sb = singles.tile([128, 1], F32, tag="zb")
            nc.vector.memset(zb_sb, 0.0)
            czero_sb = singles.tile([128, 1], F32, tag="czero")
            cone_sb = singles.tile([128, 1], F32, tag="cone")
            nc.vector.memset(czero_sb, 0.0)
            nc.vector.memset(cone_sb, 1.0)
            # memset cannot emit fp32r; DVE copies convert f32 -> f32r
            nc.vector.tensor_copy(
                out=vp_sb, in_=czero_sb.broadcast_to([128, NT, HG, 128])
            )
            for h in range(HG):
                col = 64 if h % 2 == 0 else 0
                nc.vector.tensor_copy(
                    out=vp_sb[:, :, h, col : col + 1],
                    in_=cone_sb.broadcast_to([128, NT, 1]),
                )

            def two(ap_flat):
                return ap_flat.rearrange("p (c r) -> p c r", c=2)

            # ---- QT / KT: [chtile, rows] accumulated over 8 c-chunks -------
            for w_sb, t_sb in ((wq_sb, qt_sb), (wk_sb, kt_sb)):
                for jt in range(4):
                    ps = psA.tile([128, 2, 512], F32, tag="ps")
                    for cc in range(NCC):
                        for c2 in range(2):
                            nc.tensor.matmul(
                                ps[:, c2, 0:HCH],
                                r(w_sb[:, cc, jt * 128 : (jt + 1) * 128]),
                                r(xt_sb[:, cc, c2 * HCH : (c2 + 1) * HCH]),
                                start=(cc == 0),
                                stop=(cc == NCC - 1),
                            )
                    nc.vector.tensor_copy(out=two(t_sb[:, jt, :]), in_=ps[:, :, 0:HCH])

            # ---- V: [keys, ch] row-major, scattered into vp ----------------
            for kt in range(NT):
                ps = psA.tile([128, 2, 512], F32, tag="ps")
                for cc in range(NCC):
                    nc.tensor.matmul(
                        ps[:, 0, 0:GC],
                        r(xt_sb[:, cc, kt * 128 : (kt + 1) * 128]),
                        r(wv_sb[:, cc, 0:GC]),
                        start=(cc == 0),
                        stop=(cc == NCC - 1),
                    )
                # even heads -> lhsT cols 0..63, odd heads -> cols 64..127
                pv = ps[:, 0, 0:GC].rearrange("p (h2 e c) -> p h2 e c", h2=4, e=2)
                vv = vp_sb[:, kt].rearrange("p (h2 e) m -> p h2 e m", e=2)
                nc.vector.tensor_copy(out=vv[:, :, 0:1, 0:64], in_=pv[:, :, 0:1, :])
                nc.vector.tensor_copy(out=vv[:, :, 1:2, 64:128], in_=pv[:, :, 1:2, :])

            # ---- attention, software-pipelined over (head-pair, keytile, c2)
            # Each slot: PE injects the K^T bias tile into PSUM (identity
            # matmul, start=True), the two heads' ST matmuls accumulate on top
            # (adjacent 64-partition row-groups run concurrently), one ACT exp
            # reads the PSUM pair directly into an f32r SBUF tile, and the OT
            # matmuls of the previous slot keep the PE busy while ACT works.
            ot_tiles = {}

            def step_st(j, kt, c2):
                st = psA.tile([128, 2, 512], F32, tag="ps")
                for he in range(2):
                    nc.tensor.matmul(
                        st[:, he, 0:HCH],
                        r(id_sb[:, :]),
                        r(kb_sb[:, kt, c2 * HCH : (c2 + 1) * HCH]),
                        start=True,
                        stop=False,
                    )
                for he, lo, hi in ((0, 0, 64), (1, 64, 128)):
                    nc.tensor.matmul(
                        st[:, he, 0:HCH],
                        r(kt_sb[lo:hi, j, kt * 128 : (kt + 1) * 128]),
                        r(qt_sb[lo:hi, j, c2 * HCH : (c2 + 1) * HCH]),
                        start=False,
                        stop=True,
                    )
                pt = ptpool.tile([128, 2, HCH], F32R, tag="pt")
                nc.scalar.activation(
                    out=pt, in_=st[:, :, 0:HCH],
                    func=mybir.ActivationFunctionType.Exp, bias=zb_sb[:, :],
                )
                return pt

            def step_ot(j, kt, c2, pt):
                if kt == 0 and c2 == 0:
                    ot = psB.tile([128, 2, 2, 512], F32, tag="ot")
                    ot_tiles[j] = ot
                ot = ot_tiles[j]
                for par in range(2):
                    h = 2 * j + par
                    nc.tensor.matmul(
                        ot[:, par, c2, 0:HCH],
                        r(vp_sb[:, kt, h, :]),
                        r(pt[:, par, :]),
                        start=(kt == 0),
                        stop=(kt == NT - 1),
                    )
                if kt == NT - 1 and c2 == 1:
                    finish_pair(j, ot)

            def finish_pair(j, ot):
                for par in range(2):
                    lo, hi = (0, 64) if par == 0 else (64, 128)
                    srow = 64 if par == 0 else 0
                    rc = work.tile([128, PAD], F32R, tag="rc")
                    # f32r out is bit-identical to f32; only the PE read mode
                    # differs (needed so the K=1 broadcast matmul runs 1cyc/row)
                    with nc.allow_low_precision(reason="f32r bits == f32 bits"):
                        nc.vector.reciprocal(
                            out=two(rc[srow : srow + 1, :]),
                            in_=ot[srow : srow + 1, par, :, 0:HCH],
                        )
                    # broadcast 1/sums across the 64 col partitions with a
                    # K=1 matmul against an all-ones stationary vector
                    bc = psA.tile([128, 2, 512], F32, tag="ps")
                    for c2 in range(2):
                        # matmul PSUM outputs must start at partition 0, so
                        # broadcast to all 128 partitions and slice below
                        nc.tensor.matmul(
                            bc[:, c2, 0:HCH],
                            r(on_sb[srow : srow + 1, :]),
                            r(rc[srow : srow + 1, c2 * HCH : (c2 + 1) * HCH]),
                            start=True,
                            stop=True,
                        )
                    # DVE reads at most one PSUM operand; stage bc via a DVE
                    # copy (keeps ACT pure-Exp: func switches reload tables)
                    bcs = work.tile([128, PAD], F32, tag="bcs")
                    nc.vector.tensor_copy(
                        out=two(bcs[lo:hi, :]), in_=bc[lo:hi, :, 0:HCH]
                    )
                    nc.vector.tensor_mul(
                        two(otn_sb[lo:hi, j, :]),
                        ot[lo:hi, par, :, 0:HCH],
                        two(bcs[lo:hi, :]),
                    )

            pending = []
            for j in range(n_j):
                for kt in range(NT):
                    for c2 in range(2):
                        pt = step_st(j, kt, c2)
                        pending.append((j, kt, c2, pt))
                        if len(pending) >= 2:
                            step_ot(*pending.pop(0))
            while pending:
                step_ot(*pending.pop(0))

            # ---- projection: Y[rows, 1024] ---------------------------------
            for rt in range(NT if do_y else 0):
                ps = psA.tile([128, 2, 512], F32, tag="ps")
                for oc in range(2):
                    for j in range(4):
                        nc.tensor.matmul(
                            ps[:, oc, :],
                            r(otn_sb[:, j, rt * 128 : (rt + 1) * 128]),
                            r(pw_sb[:, j, oc * 512 : (oc + 1) * 512]),
                            start=(j == 0),
                            stop=(j == 3),
                        )
                ys = work.tile([128, C], F32, tag="ys")
                nc.vector.tensor_copy(out=ys.rearrange("p (c r) -> p c r", c=2), in_=ps)
                nc.sync.dma_start(out=y_d.ap()[rt * 128 : (rt + 1) * 128, :], in_=ys)

    nc.compile()
    return nc


def _pad_for(L):
    need = -(-int(L.max()) // 128) * 128
    return max(512, need)


def _prep_inputs(PAD, x, K, n1, n2, qkv_w, qkv_b, proj_w):
    scale = np.float32(Dh**-0.5)
    L = (n1.astype(np.int64) * n2.astype(np.int64)).astype(np.int32)
    assert not np.any(qkv_b), "nonzero qkv_b not supported by this kernel"

    ident = np.eye(128, dtype=np.float16)
    onesb = np.ones((128, 128), dtype=np.float32)
    in_maps = []
    for b in range(B):
        xt = np.ascontiguousarray(x[b, :PAD, :].T).astype(np.float16)  # [C, PAD]
        # kb[key, row] = K[b, row, key] for key < L_b else -200 (exp -> 0:
        # logits are O(60) at most, so masked cols underflow exactly)
        kb = np.full((PAD, PAD), -200.0, dtype=np.float32)
        Lb = int(L[b])
        kb[:Lb, :] = K[b, :PAD, :Lb].astype(np.float32).T
        for g in range(2):
            sl = slice(g * GC, (g + 1) * GC)
            wq = np.ascontiguousarray(qkv_w[0 * C : 1 * C][sl, :].T * scale).astype(np.float16)
            wk = np.ascontiguousarray(qkv_w[1 * C : 2 * C][sl, :].T).astype(np.float16)
            wv = np.ascontiguousarray(qkv_w[2 * C : 3 * C][sl, :].T).astype(np.float16)
            pw = np.ascontiguousarray(proj_w[:, sl].T).astype(np.float16)
            in_maps.append(
                {"xt": xt, "wq": wq, "wk": wk, "wv": wv, "pw": pw,
                 "kb": kb.astype(np.float16), "ident": ident, "onesb": onesb}
            )
    return in_maps, L


def run_device(inputs, trace=False):
    """Compile (cached), run on 8 cores, return (BassKernelResults, L)."""
    from concourse import bass_utils

    x = np.asarray(inputs["x"], dtype=np.float32)
    K = np.asarray(inputs["K"], dtype=np.float32)
    n1 = np.asarray(inputs["n1"])
    n2 = np.asarray(inputs["n2"])
    L = (n1.astype(np.int64) * n2.astype(np.int64)).astype(np.int32)
    PAD = _pad_for(L)
    if ("nc", PAD) not in _CACHE:
        _CACHE[("nc", PAD)] = _build_program(PAD)
    nc = _CACHE[("nc", PAD)]

    in_maps, L = _prep_inputs(
        PAD, x, K, n1, n2,
        np.asarray(inputs["qkv_w"], dtype=np.float32),
        np.asarray(inputs["qkv_b"], dtype=np.float32),
        np.asarray(inputs["proj_w"], dtype=np.float32),
    )
    res = bass_utils.run_bass_kernel_spmd(
        nc, in_maps, core_ids=list(range(8)), trace=trace
    )
    return res, L


def kernel(**inputs):
    x = np.asarray(inputs["x"], dtype=np.float32)
    qkv_w = np.asarray(inputs["qkv_w"], dtype=np.float32)
    qkv_b = np.asarray(inputs["qkv_b"], dtype=np.float32)
    proj_w = np.asarray(inputs["proj_w"], dtype=np.float32)
    proj_b = np.asarray(inputs["proj_b"], dtype=np.float32)

    res, L = run_device(inputs)

    out = np.empty((B, N, C), dtype=np.float32)
    for b in range(B):
        Lb = int(L[b])
        yb = res.results[2 * b]["y"] + res.results[2 * b + 1]["y"] + proj_b
        out[b, :Lb] = yb[:Lb]
        # fully-masked rows: exactly uniform softmax -> mean of V
        vbar = x[b].mean(axis=0) @ qkv_w[2 * C : 3 * C, :].T + qkv_b[2 * C : 3 * C]
        out[b, Lb:] = vbar @ proj_w.T + proj_b
    return out


# revision 14
# speedup vs baseline: 1.6037x; 1.6037x over previous
# Trainium2 Bass kernel for nn_Attention_65609920413963 (sparse block-masked attention).
#
# Math structure exploited (verified against the reference numerics):
#   L_b = n1[b]*n2[b].  The reference writes NEG=-1e10 into masked logits and
#   then adds K (|K| < 1), which rounds to exactly -1e10 in fp32.  Hence:
#     * rows >= L_b: every logit is exactly -1e10 -> softmax is exactly uniform
#       -> out_row = mean(V) @ proj_w.T + proj_b  (identical for all such rows;
#       computed on host, it is O(N*C) work).
#     * rows < L_b: masked cols underflow to exp(.)=0 exactly -> softmax over
#       cols < L_b only, with additive bias K[b,row,col] on the active logits.
#   Device computes only the active [0:PAD) x [0:PAD) region (PAD >= max L,
#   multiple of 128).
#
# Sharding: 8 cores = (batch b in 0..3) x (head-half g in 0..1, 8 heads each).
# Per-core device pipeline (all matmuls fp32r):
#   QT/KT  [ch, rows]   = Wq/Wk.T @ x.T          (feature-major)
#   V      [keys, ch]   = x @ Wv                 (row-major)
#   ST_h   [keys, rows] = KB + K8_h @ Q_h.T      (KB = K^T with masked cols at
#                                                 -200, injected into PSUM by an
#                                                 identity matmul so the ST
#                                                 accumulation starts from the
#                                                 bias; masked cols underflow to
#                                                 exp 0 with no extra masking)
#   PT_h   = exp(ST_h)                           (ACT reads PSUM directly)
#   OT_h   [.., rows]   = [V_h | ones].T @ PT_h  (ones column -> partition 64/0
#                                                 carries the softmax denominators)
#   OTn_h  = OT_h * (1/denominator)              (1/den broadcast across the 64
#                                                 col partitions by a K=1 matmul
#                                                 against an all-ones lhsT)
#   Y      [rows, 1024] = OTn @ proj_w_g         (partial product; host adds the
#                                                 two head-halves + proj_b)
import numpy as np

B, N, C = 4, 1024, 1024
H, Dh = 16, 64
HG = H // 2          # heads per core
GC = HG * Dh         # channels per core (512)
NCC = C // 128       # 8 contraction chunks

_CACHE = {}


def _build_program(PAD, reps=1, n_j=4, do_y=True):
    import concourse.bacc as bacc
    import concourse.bass as bass
    import concourse.mybir as mybir
    import concourse.tile as tile

    NT = PAD // 128
    HCH = PAD // 2    # psum half-chunk of the row dimension (<=512, >=256)
    assert 256 <= HCH <= 512

    F32 = mybir.dt.float32
    F32R = mybir.dt.float32r
    F16 = mybir.dt.float16

    nc = bacc.Bacc("TRN2", target_bir_lowering=False, debug=False)

    xt_d = nc.dram_tensor("xt", [C, PAD], F32R, kind="ExternalInput")
    wq_d = nc.dram_tensor("wq", [C, GC], F32R, kind="ExternalInput")
    wk_d = nc.dram_tensor("wk", [C, GC], F32R, kind="ExternalInput")
    wv_d = nc.dram_tensor("wv", [C, GC], F32R, kind="ExternalInput")
    pw_d = nc.dram_tensor("pw", [GC, C], F32R, kind="ExternalInput")
    kb_d = nc.dram_tensor("kb", [PAD, PAD], F32R, kind="ExternalInput")
    id_d = nc.dram_tensor("ident", [128, 128], F32R, kind="ExternalInput")
    on_d = nc.dram_tensor("onesb", [128, 128], F32R, kind="ExternalInput")
    y_d = nc.dram_tensor("y", [PAD, C], F32, kind="ExternalOutput")

    def r(ap):
        return ap

    import contextlib

    with tile.TileContext(nc) as tc:
        with (
            tc.For_i(0, reps, 1) if reps > 1 else contextlib.nullcontext(),
            tc.tile_pool(name="singles", bufs=1) as singles,
            tc.tile_pool(name="wpool", bufs=2) as wpool,
            tc.tile_pool(name="work", bufs=3) as work,
            tc.tile_pool(name="ptpool", bufs=4) as ptpool,
            tc.tile_pool(name="psA", bufs=2, space="PSUM") as psA,
            tc.tile_pool(name="psB", bufs=1, space="PSUM") as psB,
        ):
            # ---- resident SBUF tensors -------------------------------------
            xt_sb = singles.tile([128, NCC, PAD], F32R, tag="xt")
            # wq/wk/wv share 2 slots: wv reuses wq's slot once QT is done
            wq_sb = wpool.tile([128, NCC, GC], F32R, tag="w")
            wk_sb = wpool.tile([128, NCC, GC], F32R, tag="w")
            wv_sb = wpool.tile([128, NCC, GC], F32R, tag="w")
            pw_sb = singles.tile([128, 4, C], F32R, tag="pw")
            kb_sb = singles.tile([128, NT, PAD], F32R, tag="kb")
            id_sb = singles.tile([128, 128], F32R, tag="id")
            on_sb = singles.tile([128, 128], F32R, tag="on")
            qt_sb = singles.tile([128, 4, PAD], F32R, tag="qt")
            kt_sb = singles.tile([128, 4, PAD], F32R, tag="kt")
            vp_sb = singles.tile([128, NT, HG, 128], F32R, tag="vp")
            otn_sb = singles.tile([128, 4, PAD], F32R, tag="otn")

            # per-contraction-chunk DMAs, interleaved so the first QT matmuls
            # start after ~0.7MB instead of the full 5MB of xt+wq
            xt_r = xt_d.ap().rearrange("(a p) r -> p a r", p=128)
            wq_r = wq_d.ap().rearrange("(a p) m -> p a m", p=128)
            wk_r = wk_d.ap().rearrange("(a p) m -> p a m", p=128)
            for cc in range(NCC):
                nc.sync.dma_start(out=xt_sb[:, cc], in_=xt_r[:, cc])
                nc.sync.dma_start(out=wq_sb[:, cc], in_=wq_r[:, cc])
            for cc in range(NCC):
                nc.sync.dma_start(out=wk_sb[:, cc], in_=wk_r[:, cc])
            nc.sync.dma_start(out=wv_sb, in_=wv_d.ap().rearrange("(a p) m -> p a m", p=128))
            nc.sync.dma_start(out=kb_sb, in_=kb_d.ap().rearrange("(t p) r -> p t r", p=128))
            nc.sync.dma_start(out=id_sb, in_=id_d.ap())
            nc.sync.dma_start(out=on_sb, in_=on_d.ap())
            nc.sync.dma_start(out=pw_sb, in_=pw_d.ap().rearrange("(j p) o -> p j o", p=128))

            # vp: [keypart, keytile, head, Mcol].  even h: cols 0..63 = V_h, col 64 = 1
            #                                      odd  h: col 0 = 1, cols 64..127 = V_h
            zb_sb = singles.tile([128, 1], F32, tag="zb")
            nc.vector.memset(zb_sb, 0.0)
            czero_sb = singles.tile([128, 1], F32, tag="czero")
            cone_sb = singles.tile([128, 1], F32, tag="cone")
            nc.vector.memset(czero_sb, 0.0)
            nc.vector.memset(cone_sb, 1.0)
            # memset cannot emit fp32r; DVE copies convert f32 -> f32r
            nc.vector.tensor_copy(
                out=vp_sb, in_=czero_sb.broadcast_to([128, NT, HG, 128])
            )
            for h in range(HG):
                col = 64 if h % 2 == 0 else 0
                nc.vector.tensor_copy(
                    out=vp_sb[:, :, h, col : col + 1],
                    in_=cone_sb.broadcast_to([128, NT, 1]),
                )

            def two(ap_flat):
                return ap_flat.rearrange("p (c r) -> p c r", c=2)

            # ---- QT / KT: [chtile, rows] accumulated over 8 c-chunks -------
            for w_sb, t_sb in ((wq_sb, qt_sb), (wk_sb, kt_sb)):
                for jt in range(4):
                    ps = psA.tile([128, 2, 512], F32, tag="ps")
                    for cc in range(NCC):
                        for c2 in range(2):
                            nc.tensor.matmul(
                                ps[:, c2, 0:HCH],
                                r(w_sb[:, cc, jt * 128 : (jt + 1) * 128]),
                                r(xt_sb[:, cc, c2 * HCH : (c2 + 1) * HCH]),
                                start=(cc == 0),
                                stop=(cc == NCC - 1),
                            )
                    nc.vector.tensor_copy(out=two(t_sb[:, jt, :]), in_=ps[:, :, 0:HCH])

            # ---- V: [keys, ch] row-major, scattered into vp ----------------
            for kt in range(NT):
                ps = psA.tile([128, 2, 512], F32, tag="ps")
                for cc in range(NCC):
                    nc.tensor.matmul(
                        ps[:, 0, 0:GC],
                        r(xt_sb[:, cc, kt * 128 : (kt + 1) * 128]),
                        r(wv_sb[:, cc, 0:GC]),
                        start=(cc == 0),
                        stop=(cc == NCC - 1),
                    )
                # even heads -> lhsT cols 0..63, odd heads -> cols 64..127
                pv = ps[:, 0, 0:GC].rearrange("p (h2 e c) -> p h2 e c", h2=4, e=2)
                vv = vp_sb[:, kt].rearrange("p (h2 e) m -> p h2 e m", e=2)
                nc.vector.tensor_copy(out=vv[:, :, 0:1, 0:64], in_=pv[:, :, 0:1, :])
                nc.vector.tensor_copy(out=vv[:, :, 1:2, 64:128], in_=pv[:, :, 1:2, :])

            # ---- attention, software-pipelined over (head-pair, keytile, c2)
            # Each slot: PE injects the K^T bias tile into PSUM (identity
            # matmul, start=True), the two heads' ST matmuls accumulate on top
            # (adjacent 64-partition row-groups run concurrently), one ACT exp
            # reads the PSUM pair directly into an f32r SBUF tile, and the OT
            # matmuls of the previous slot keep the PE busy while ACT works.
            ot_tiles = {}

            def step_st(j, kt, c2):
                st = psA.tile([128, 2, 512], F32, tag="ps")
                for he in range(2):
                    nc.tensor.matmul(
                        st[:, he, 0:HCH],
                        r(id_sb[:, :]),
                        r(kb_sb[:, kt, c2 * HCH : (c2 + 1) * HCH]),
                        start=True,
                        stop=False,
                    )
                for he, lo, hi in ((0, 0, 64), (1, 64, 128)):
                    nc.tensor.matmul(
                        st[:, he, 0:HCH],
                        r(kt_sb[lo:hi, j, kt * 128 : (kt + 1) * 128]),
                        r(qt_sb[lo:hi, j, c2 * HCH : (c2 + 1) * HCH]),
                        start=False,
                        stop=True,
                    )
                pt = ptpool.tile([128, 2, HCH], F32R, tag="pt")
                nc.scalar.activation(
                    out=pt, in_=st[:, :, 0:HCH],
                    func=mybir.ActivationFunctionType.Exp, bias=zb_sb[:, :],
                )
                return pt

            def step_ot(j, kt, c2, pt):
                if kt == 0 and c2 == 0:
                    ot = psB.tile([128, 2, 2, 512], F32, tag="ot")
                    ot_tiles[j] = ot
                ot = ot_tiles[j]
                for par in range(2):
                    h = 2 * j + par
                    nc.tensor.matmul(
                        ot[:, par, c2, 0:HCH],
                        r(vp_sb[:, kt, h, :]),
                        r(pt[:, par, :]),
                        start=(kt == 0),
                        stop=(kt == NT - 1),
                    )

            def finish_half(j, c2):
                # softmax denominators for the c2 row-half of head pair j:
                # reciprocal on DVE, partition-broadcast via a K=1 matmul
                # against all-ones, stage through SBUF, then normalize.
                ot = ot_tiles[j]
                hsl = slice(c2 * HCH, (c2 + 1) * HCH)
                rc = work.tile([128, PAD], F32R, tag="rc")
                bc = psA.tile([128, 2, 512], F32, tag="ps")
                for par, lo, hi, srow in ((0, 0, 64, 64), (1, 64, 128, 0)):
                    # f32r out is bit-identical to f32; only the PE read mode
                    # differs (the K=1 broadcast matmul then runs 1cyc/row)
                    with nc.allow_low_precision(reason="f32r bits == f32 bits"):
                        nc.vector.reciprocal(
                            out=rc[srow : srow + 1, hsl],
                            in_=ot[srow : srow + 1, par, c2, 0:HCH],
                        )
                    # matmul PSUM outputs must start at partition 0, so
                    # broadcast to all 128 partitions and slice below
                    nc.tensor.matmul(
                        bc[:, par, 0:HCH],
                        r(on_sb[srow : srow + 1, :]),
                        r(rc[srow : srow + 1, hsl]),
                        start=True,
                        stop=True,
                    )
                # DVE reads at most one PSUM operand; stage bc via a DVE copy
                bcs = work.tile([128, PAD], F32, tag="bcs")
                for par, lo, hi, srow in ((0, 0, 64, 64), (1, 64, 128, 0)):
                    nc.vector.tensor_copy(
                        out=bcs[lo:hi, hsl], in_=bc[lo:hi, par, 0:HCH]
                    )
                    nc.vector.tensor_mul(
                        otn_sb[lo:hi, j, hsl],
                        ot[lo:hi, par, c2, 0:HCH],
                        bcs[lo:hi, hsl],
                    )

            # software pipeline: OT trails ST by one slot, and each (j, c2)
            # row-half's normalization is deferred one further slot so the PE
            # has fresh inject/ST work while DVE computes the reciprocals
            pending = []
            finishq = []
            for j in range(n_j):
                for kt in range(NT):
                    for c2 in range(2):
                        pt = step_st(j, kt, c2)
                        pending.append((j, kt, c2, pt))
                        while finishq:
                            finish_half(*finishq.pop(0))
                        if len(pending) >= 2:
                            jo, ko, co, po = pending.pop(0)
                            step_ot(jo, ko, co, po)
                            if ko == NT - 1:
                                finishq.append((jo, co))
            while pending:
                jo, ko, co, po = pending.pop(0)
                step_ot(jo, ko, co, po)
                if ko == NT - 1:
                    finishq.append((jo, co))
            while finishq:
                finish_half(*finishq.pop(0))

            # ---- projection: Y[rows, 1024] ---------------------------------
            for rt in range(NT if do_y else 0):
                ps = psA.tile([128, 2, 512], F32, tag="ps")
                for oc in range(2):
                    for j in range(4):
                        nc.tensor.matmul(
                            ps[:, oc, :],
                            r(otn_sb[:, j, rt * 128 : (rt + 1) * 128]),
                            r(pw_sb[:, j, oc * 512 : (oc + 1) * 512]),
                            start=(j == 0),
                            stop=(j == 3),
                        )
                ys = work.tile([128, C], F32, tag="ys")
                nc.vector.tensor_copy(out=ys.rearrange("p (c r) -> p c r", c=2), in_=ps)
                nc.sync.dma_start(out=y_d.ap()[rt * 128 : (rt + 1) * 128, :], in_=ys)

    nc.compile()
    return nc


def _pad_for(L):
    need = -(-int(L.max()) // 128) * 128
    return max(512, need)


def _prep_inputs(PAD, x, K, n1, n2, qkv_w, qkv_b, proj_w):
    scale = np.float32(Dh**-0.5)
    L = (n1.astype(np.int64) * n2.astype(np.int64)).astype(np.int32)
    assert not np.any(qkv_b), "nonzero qkv_b not supported by this kernel"

    ident = np.eye(128, dtype=np.float32)
    onesb = np.ones((128, 128), dtype=np.float32)
    in_maps = []
    for b in range(B):
        xt = np.ascontiguousarray(x[b, :PAD, :].T)  # [C, PAD]
        # kb[key, row] = K[b, row, key] for key < L_b else -200 (exp -> 0:
        # logits are O(60) at most, so masked cols underflow exactly)
        kb = np.full((PAD, PAD), -200.0, dtype=np.float32)
        Lb = int(L[b])
        kb[:Lb, :] = K[b, :PAD, :Lb].astype(np.float32).T
        for g in range(2):
            sl = slice(g * GC, (g + 1) * GC)
            wq = np.ascontiguousarray(qkv_w[0 * C : 1 * C][sl, :].T * scale)
            wk = np.ascontiguousarray(qkv_w[1 * C : 2 * C][sl, :].T)
            wv = np.ascontiguousarray(qkv_w[2 * C : 3 * C][sl, :].T)
            pw = np.ascontiguousarray(proj_w[:, sl].T)
            in_maps.append(
                {"xt": xt, "wq": wq, "wk": wk, "wv": wv, "pw": pw,
                 "kb": kb, "ident": ident, "onesb": onesb}
            )
    return in_maps, L


def run_device(inputs, trace=False):
    """Compile (cached), run on 8 cores, return (BassKernelResults, L)."""
    from concourse import bass_utils

    x = np.asarray(inputs["x"], dtype=np.float32)
    K = np.asarray(inputs["K"], dtype=np.float32)
    n1 = np.asarray(inputs["n1"])
    n2 = np.asarray(inputs["n2"])
    L = (n1.astype(np.int64) * n2.astype(np.int64)).astype(np.int32)
    PAD = _pad_for(L)
    if ("nc", PAD) not in _CACHE:
        _CACHE[("nc", PAD)] = _build_program(PAD)
    nc = _CACHE[("nc", PAD)]

    in_maps, L = _prep_inputs(
        PAD, x, K, n1, n2,
        np.asarray(inputs["qkv_w"], dtype=np.float32),
        np.asarray(inputs["qkv_b"], dtype=np.float32),
        np.asarray(inputs["proj_w"], dtype=np.float32),
    )
    res = bass_utils.run_bass_kernel_spmd(
        nc, in_maps, core_ids=list(range(8)), trace=trace
    )
    return res, L


def kernel(**inputs):
    x = np.asarray(inputs["x"], dtype=np.float32)
    qkv_w = np.asarray(inputs["qkv_w"], dtype=np.float32)
    qkv_b = np.asarray(inputs["qkv_b"], dtype=np.float32)
    proj_w = np.asarray(inputs["proj_w"], dtype=np.float32)
    proj_b = np.asarray(inputs["proj_b"], dtype=np.float32)

    res, L = run_device(inputs)

    out = np.empty((B, N, C), dtype=np.float32)
    for b in range(B):
        Lb = int(L[b])
        yb = res.results[2 * b]["y"] + res.results[2 * b + 1]["y"] + proj_b
        out[b, :Lb] = yb[:Lb]
        # fully-masked rows: exactly uniform softmax -> mean of V
        vbar = x[b].mean(axis=0) @ qkv_w[2 * C : 3 * C, :].T + qkv_b[2 * C : 3 * C]
        out[b, Lb:] = vbar @ proj_w.T + proj_b
    return out


# revision 22
# speedup vs baseline: 1.9340x; 1.2059x over previous
# Trainium2 Bass kernel for nn_Attention_65609920413963 (sparse block-masked attention).
#
# Math structure exploited (verified against the reference numerics):
#   L_b = n1[b]*n2[b].  The reference writes NEG=-1e10 into masked logits and
#   then adds K (|K| < 1), which rounds to exactly -1e10 in fp32.  Hence:
#     * rows >= L_b: every logit is exactly -1e10 -> softmax is exactly uniform
#       -> out_row = mean(V) @ proj_w.T + proj_b  (identical for all such rows;
#       computed on host, it is O(N*C) work).
#     * rows < L_b: masked cols underflow to exp(.)=0 exactly -> softmax over
#       cols < L_b only, with additive bias K[b,row,col] on the active logits.
#   Device computes only the active [0:PAD) x [0:PAD) region (PAD >= max L,
#   multiple of 128).
#
# Sharding: 8 cores = (batch b in 0..3) x (head-half g in 0..1, 8 heads each).
# Per-core device pipeline (all matmuls fp32r):
#   QT/KT  [ch, rows]   = Wq/Wk.T @ x.T          (feature-major)
#   V      [keys, ch]   = x @ Wv                 (row-major)
#   ST_h   [keys, rows] = KB + K8_h @ Q_h.T      (KB = K^T with masked cols at
#                                                 -200, injected into PSUM by an
#                                                 identity matmul so the ST
#                                                 accumulation starts from the
#                                                 bias; masked cols underflow to
#                                                 exp 0 with no extra masking)
#   PT_h   = exp(ST_h)                           (ACT reads PSUM directly)
#   OT_h   [.., rows]   = [V_h | ones].T @ PT_h  (ones column -> partition 64/0
#                                                 carries the softmax denominators)
#   OTn_h  = OT_h * (1/denominator)              (1/den broadcast across the 64
#                                                 col partitions by a K=1 matmul
#                                                 against an all-ones lhsT)
#   Y      [rows, 1024] = OTn @ proj_w_g         (partial product; host adds the
#                                                 two head-halves + proj_b)
import numpy as np

B, N, C = 4, 1024, 1024
H, Dh = 16, 64
HG = H // 2          # heads per core
GC = HG * Dh         # channels per core (512)
NCC = C // 128       # 8 contraction chunks

_CACHE = {}


def _build_program(PAD, reps=1, n_j=4, do_y=True, dma_mode="pool_f32", do_qkv=True, ring_plan="a", y_f16=True):
    import concourse.bacc as bacc
    import concourse.bass as bass
    import concourse.mybir as mybir
    import concourse.tile as tile

    NT = PAD // 128
    HCH = PAD // 2    # psum half-chunk of the row dimension (<=512, >=256)
    assert 256 <= HCH <= 512

    F32 = mybir.dt.float32
    F32R = mybir.dt.float32r
    F16 = mybir.dt.float16

    nc = bacc.Bacc("TRN2", target_bir_lowering=False, debug=False)

    WDT = F16 if dma_mode == "pool_f16" else F32R
    xt_d = nc.dram_tensor("xt", [C, PAD], F32R, kind="ExternalInput")
    wq_d = nc.dram_tensor("wq", [C, GC], WDT, kind="ExternalInput")
    wk_d = nc.dram_tensor("wk", [C, GC], WDT, kind="ExternalInput")
    wv_d = nc.dram_tensor("wv", [C, GC], WDT, kind="ExternalInput")
    pw_d = nc.dram_tensor("pw", [GC, C], WDT, kind="ExternalInput")
    kb_d = nc.dram_tensor("kb", [PAD, PAD], WDT, kind="ExternalInput")
    id_d = nc.dram_tensor("ident", [128, 128], F32R, kind="ExternalInput")
    on_d = nc.dram_tensor("onesb", [128, 128], F32R, kind="ExternalInput")
    y_d = nc.dram_tensor("y", [PAD, C], F16 if y_f16 else F32, kind="ExternalOutput")

    def r(ap):
        return ap

    import contextlib

    with tile.TileContext(nc) as tc:
        with (
            tc.For_i(0, reps, 1) if reps > 1 else contextlib.nullcontext(),
            tc.tile_pool(name="singles", bufs=1) as singles,
            tc.tile_pool(name="wpool", bufs=2) as wpool,
            tc.tile_pool(name="work", bufs=3) as work,
            tc.tile_pool(name="ptpool", bufs=4) as ptpool,
            tc.tile_pool(name="psA", bufs=2, space="PSUM") as psA,
            tc.tile_pool(name="psB", bufs=1, space="PSUM") as psB,
        ):
            # ---- resident SBUF tensors -------------------------------------
            xt_sb = singles.tile([128, NCC, PAD], F32R, tag="xt")
            # wq/wk/wv share 2 slots: wv reuses wq's slot once QT is done
            wq_sb = wpool.tile([128, NCC, GC], F32R, tag="w")
            wk_sb = wpool.tile([128, NCC, GC], F32R, tag="w")
            wv_sb = wpool.tile([128, NCC, GC], F32R, tag="w")
            pw_sb = singles.tile([128, 4, C], F32R, tag="pw")
            kb_sb = singles.tile([128, NT, PAD], F32R, tag="kb")
            id_sb = singles.tile([128, 128], F32R, tag="id")
            on_sb = singles.tile([128, 128], F32R, tag="on")
            qt_sb = singles.tile([128, 4, PAD], F32R, tag="qt")
            kt_sb = singles.tile([128, 4, PAD], F32R, tag="kt")
            vp_sb = singles.tile([128, NT, HG, 128], F32R, tag="vp")
            otn_sb = singles.tile([128, 4, PAD], F32R, tag="otn")

            # Input streaming: the SP HWDGE ring serializes at ~70-80 GB/s,
            # so 12MB of f32 inputs alone would take ~150us/rep.  Split the
            # load across the otherwise-idle gpsimd SWDGE ring (which can
            # also CAST fp16 dram -> f32r sbuf, halving wire bytes).  The ACT
            # HWDGE ring is NOT used: its dma_starts block the ACT sequencer
            # that must run the exps.
            q2 = nc.sync if dma_mode == "sp" else nc.gpsimd
            qxt = q2 if ring_plan == "d" else nc.sync
            xt_r = xt_d.ap().rearrange("(a p) r -> p a r", p=128)
            wq_r = wq_d.ap().rearrange("(a p) m -> p a m", p=128)
            wk_r = wk_d.ap().rearrange("(a p) m -> p a m", p=128)
            for cc in range(NCC):
                qxt.dma_start(out=xt_sb[:, cc], in_=xt_r[:, cc])
                q2.dma_start(out=wq_sb[:, cc], in_=wq_r[:, cc])
            for cc in range(NCC):
                q2.dma_start(out=wk_sb[:, cc], in_=wk_r[:, cc])
            wv_r = wv_d.ap().rearrange("(a p) m -> p a m", p=128)
            for cc in range(NCC):
                q2.dma_start(out=wv_sb[:, cc], in_=wv_r[:, cc])
            kb_r = kb_d.ap().rearrange("(t p) r -> p t r", p=128)
            for kt in range(NT):
                q2.dma_start(out=kb_sb[:, kt], in_=kb_r[:, kt])
            nc.sync.dma_start(out=id_sb, in_=id_d.ap())
            nc.sync.dma_start(out=on_sb, in_=on_d.ap())
            q2.dma_start(out=pw_sb, in_=pw_d.ap().rearrange("(j p) o -> p j o", p=128))

            # vp: [keypart, keytile, head, Mcol].  even h: cols 0..63 = V_h, col 64 = 1
            #                                      odd  h: col 0 = 1, cols 64..127 = V_h
            zb_sb = singles.tile([128, 1], F32, tag="zb")
            nc.vector.memset(zb_sb, 0.0)
            czero_sb = singles.tile([128, 1], F32, tag="czero")
            cone_sb = singles.tile([128, 1], F32, tag="cone")
            nc.vector.memset(czero_sb, 0.0)
            nc.vector.memset(cone_sb, 1.0)
            # memset cannot emit fp32r; DVE copies convert f32 -> f32r
            nc.vector.tensor_copy(
                out=vp_sb, in_=czero_sb.broadcast_to([128, NT, HG, 128])
            )
            for h in range(HG):
                col = 64 if h % 2 == 0 else 0
                nc.vector.tensor_copy(
                    out=vp_sb[:, :, h, col : col + 1],
                    in_=cone_sb.broadcast_to([128, NT, 1]),
                )

            def two(ap_flat):
                return ap_flat.rearrange("p (c r) -> p c r", c=2)

            # ---- QT / KT: [chtile, rows] accumulated over 8 c-chunks -------
            for w_sb, t_sb in (((wq_sb, qt_sb), (wk_sb, kt_sb)) if do_qkv else ()):
                for jt in range(4):
                    ps = psA.tile([128, 2, 512], F32, tag="ps")
                    for cc in range(NCC):
                        for c2 in range(2):
                            nc.tensor.matmul(
                                ps[:, c2, 0:HCH],
                                r(w_sb[:, cc, jt * 128 : (jt + 1) * 128]),
                                r(xt_sb[:, cc, c2 * HCH : (c2 + 1) * HCH]),
                                start=(cc == 0),
                                stop=(cc == NCC - 1),
                            )
                    nc.vector.tensor_copy(out=two(t_sb[:, jt, :]), in_=ps[:, :, 0:HCH])

            # ---- V: [keys, ch] row-major, scattered into vp ----------------
            for kt in range(NT if do_qkv else 0):
                ps = psA.tile([128, 2, 512], F32, tag="ps")
                for cc in range(NCC):
                    nc.tensor.matmul(
                        ps[:, 0, 0:GC],
                        r(xt_sb[:, cc, kt * 128 : (kt + 1) * 128]),
                        r(wv_sb[:, cc, 0:GC]),
                        start=(cc == 0),
                        stop=(cc == NCC - 1),
                    )
                # even heads -> lhsT cols 0..63, odd heads -> cols 64..127
                pv = ps[:, 0, 0:GC].rearrange("p (h2 e c) -> p h2 e c", h2=4, e=2)
                vv = vp_sb[:, kt].rearrange("p (h2 e) m -> p h2 e m", e=2)
                nc.vector.tensor_copy(out=vv[:, :, 0:1, 0:64], in_=pv[:, :, 0:1, :])
                nc.vector.tensor_copy(out=vv[:, :, 1:2, 64:128], in_=pv[:, :, 1:2, :])

            # ---- attention, software-pipelined over (head-pair, keytile, c2)
            # Each slot: PE injects the K^T bias tile into PSUM (identity
            # matmul, start=True), the two heads' ST matmuls accumulate on top
            # (adjacent 64-partition row-groups run concurrently), one ACT exp
            # reads the PSUM pair directly into an f32r SBUF tile, and the OT
            # matmuls of the previous slot keep the PE busy while ACT works.
            ot_tiles = {}

            def step_st(j, kt, c2):
                st = psA.tile([128, 2, 512], F32, tag="ps")
                for he in range(2):
                    nc.tensor.matmul(
                        st[:, he, 0:HCH],
                        r(id_sb[:, :]),
                        r(kb_sb[:, kt, c2 * HCH : (c2 + 1) * HCH]),
                        start=True,
                        stop=False,
                    )
                for he, lo, hi in ((0, 0, 64), (1, 64, 128)):
                    nc.tensor.matmul(
                        st[:, he, 0:HCH],
                        r(kt_sb[lo:hi, j, kt * 128 : (kt + 1) * 128]),
                        r(qt_sb[lo:hi, j, c2 * HCH : (c2 + 1) * HCH]),
                        start=False,
                        stop=True,
                    )
                pt = ptpool.tile([128, 2, HCH], F32R, tag="pt")
                nc.scalar.activation(
                    out=pt, in_=st[:, :, 0:HCH],
                    func=mybir.ActivationFunctionType.Exp, bias=zb_sb[:, :],
                )
                return pt

            def step_ot(j, kt, c2, pt):
                if kt == 0 and c2 == 0:
                    ot = psB.tile([128, 2, 2, 512], F32, tag="ot")
                    ot_tiles[j] = ot
                ot = ot_tiles[j]
                for par in range(2):
                    h = 2 * j + par
                    nc.tensor.matmul(
                        ot[:, par, c2, 0:HCH],
                        r(vp_sb[:, kt, h, :]),
                        r(pt[:, par, :]),
                        start=(kt == 0),
                        stop=(kt == NT - 1),
                    )

            def finish_half(j, c2):
                # softmax denominators for the c2 row-half of head pair j:
                # reciprocal on DVE, partition-broadcast via a K=1 matmul
                # against all-ones, stage through SBUF, then normalize.
                ot = ot_tiles[j]
                hsl = slice(c2 * HCH, (c2 + 1) * HCH)
                rc = work.tile([128, PAD], F32R, tag="rc")
                bc = psA.tile([128, 2, 512], F32, tag="ps")
                for par, lo, hi, srow in ((0, 0, 64, 64), (1, 64, 128, 0)):
                    # f32r out is bit-identical to f32; only the PE read mode
                    # differs (the K=1 broadcast matmul then runs 1cyc/row)
                    with nc.allow_low_precision(reason="f32r bits == f32 bits"):
                        nc.vector.reciprocal(
                            out=rc[srow : srow + 1, hsl],
                            in_=ot[srow : srow + 1, par, c2, 0:HCH],
                        )
                    # matmul PSUM outputs must start at partition 0, so
                    # broadcast to all 128 partitions and slice below
                    nc.tensor.matmul(
                        bc[:, par, 0:HCH],
                        r(on_sb[srow : srow + 1, :]),
                        r(rc[srow : srow + 1, hsl]),
                        start=True,
                        stop=True,
                    )
                # DVE reads at most one PSUM operand; stage bc via a DVE copy
                bcs = work.tile([128, PAD], F32, tag="bcs")
                for par, lo, hi, srow in ((0, 0, 64, 64), (1, 64, 128, 0)):
                    nc.vector.tensor_copy(
                        out=bcs[lo:hi, hsl], in_=bc[lo:hi, par, 0:HCH]
                    )
                    nc.vector.tensor_mul(
                        otn_sb[lo:hi, j, hsl],
                        ot[lo:hi, par, c2, 0:HCH],
                        bcs[lo:hi, hsl],
                    )

            # software pipeline: OT trails ST by one slot, and each (j, c2)
            # row-half's normalization is deferred one further slot so the PE
            # has fresh inject/ST work while DVE computes the reciprocals
            pending = []
            finishq = []
            for j in range(n_j):
                for kt in range(NT):
                    for c2 in range(2):
                        pt = step_st(j, kt, c2)
                        pending.append((j, kt, c2, pt))
                        while finishq:
                            finish_half(*finishq.pop(0))
                        if len(pending) >= 2:
                            jo, ko, co, po = pending.pop(0)
                            step_ot(jo, ko, co, po)
                            if ko == NT - 1:
                                finishq.append((jo, co))
            while pending:
                jo, ko, co, po = pending.pop(0)
                step_ot(jo, ko, co, po)
                if ko == NT - 1:
                    finishq.append((jo, co))
            while finishq:
                finish_half(*finishq.pop(0))

            # ---- projection: Y[rows, 1024] ---------------------------------
            for rt in range(NT if do_y else 0):
                ps = psA.tile([128, 2, 512], F32, tag="ps")
                for oc in range(2):
                    for j in range(4):
                        nc.tensor.matmul(
                            ps[:, oc, :],
                            r(otn_sb[:, j, rt * 128 : (rt + 1) * 128]),
                            r(pw_sb[:, j, oc * 512 : (oc + 1) * 512]),
                            start=(j == 0),
                            stop=(j == 3),
                        )
                ys = work.tile([128, C], F16 if y_f16 else F32, tag="ys")
                with nc.allow_low_precision(reason="fp16 output staging, tol 2e-2"):
                    nc.vector.tensor_copy(out=ys.rearrange("p (c r) -> p c r", c=2), in_=ps)
                qy = nc.scalar if ring_plan == "f" else nc.sync
                qy.dma_start(out=y_d.ap()[rt * 128 : (rt + 1) * 128, :], in_=ys)

    nc.compile()
    return nc


def _pad_for(L):
    need = -(-int(L.max()) // 128) * 128
    return max(512, need)


def _prep_inputs(PAD, x, K, n1, n2, qkv_w, qkv_b, proj_w, dma_mode="pool_f32"):
    scale = np.float32(Dh**-0.5)
    L = (n1.astype(np.int64) * n2.astype(np.int64)).astype(np.int32)
    assert not np.any(qkv_b), "nonzero qkv_b not supported by this kernel"

    wdt = np.float16 if dma_mode == "pool_f16" else np.float32
    ident = np.eye(128, dtype=np.float32)
    onesb = np.ones((128, 128), dtype=np.float32)
    in_maps = []
    for b in range(B):
        xt = np.ascontiguousarray(x[b, :PAD, :].T)  # [C, PAD]
        # kb[key, row] = K[b, row, key] for key < L_b else -200 (exp -> 0:
        # logits are O(60) at most, so masked cols underflow exactly)
        kb = np.full((PAD, PAD), -200.0, dtype=np.float32)
        Lb = int(L[b])
        kb[:Lb, :] = K[b, :PAD, :Lb].astype(np.float32).T
        for g in range(2):
            sl = slice(g * GC, (g + 1) * GC)
            wq = np.ascontiguousarray(qkv_w[0 * C : 1 * C][sl, :].T * scale).astype(wdt)
            wk = np.ascontiguousarray(qkv_w[1 * C : 2 * C][sl, :].T).astype(wdt)
            wv = np.ascontiguousarray(qkv_w[2 * C : 3 * C][sl, :].T).astype(wdt)
            pw = np.ascontiguousarray(proj_w[:, sl].T).astype(wdt)
            in_maps.append(
                {"xt": xt, "wq": wq, "wk": wk, "wv": wv, "pw": pw,
                 "kb": kb.astype(wdt), "ident": ident, "onesb": onesb}
            )
    return in_maps, L


def run_device(inputs, trace=False):
    """Compile (cached), run on 8 cores, return (BassKernelResults, L)."""
    from concourse import bass_utils

    x = np.asarray(inputs["x"], dtype=np.float32)
    K = np.asarray(inputs["K"], dtype=np.float32)
    n1 = np.asarray(inputs["n1"])
    n2 = np.asarray(inputs["n2"])
    L = (n1.astype(np.int64) * n2.astype(np.int64)).astype(np.int32)
    PAD = _pad_for(L)
    if ("nc", PAD) not in _CACHE:
        _CACHE[("nc", PAD)] = _build_program(PAD)
    nc = _CACHE[("nc", PAD)]

    in_maps, L = _prep_inputs(
        PAD, x, K, n1, n2,
        np.asarray(inputs["qkv_w"], dtype=np.float32),
        np.asarray(inputs["qkv_b"], dtype=np.float32),
        np.asarray(inputs["proj_w"], dtype=np.float32),
    )
    res = bass_utils.run_bass_kernel_spmd(
        nc, in_maps, core_ids=list(range(8)), trace=trace
    )
    return res, L


def kernel(**inputs):
    x = np.asarray(inputs["x"], dtype=np.float32)
    qkv_w = np.asarray(inputs["qkv_w"], dtype=np.float32)
    qkv_b = np.asarray(inputs["qkv_b"], dtype=np.float32)
    proj_w = np.asarray(inputs["proj_w"], dtype=np.float32)
    proj_b = np.asarray(inputs["proj_b"], dtype=np.float32)

    res, L = run_device(inputs)

    out = np.empty((B, N, C), dtype=np.float32)
    for b in range(B):
        Lb = int(L[b])
        yb = (res.results[2 * b]["y"].astype(np.float32)
              + res.results[2 * b + 1]["y"].astype(np.float32) + proj_b)
        out[b, :Lb] = yb[:Lb]
        # fully-masked rows: exactly uniform softmax -> mean of V
        vbar = x[b].mean(axis=0) @ qkv_w[2 * C : 3 * C, :].T + qkv_b[2 * C : 3 * C]
        out[b, Lb:] = vbar @ proj_w.T + proj_b
    return out
